# revision 1
# baseline (speedup 1.0000x reference)
"""Gated MLP (SwiGLU) on 8 TRN2 NeuronCores, tensor-parallel over the
intermediate dimension.

Math (per reference): g = x @ Wg.T ; u = x @ Wu.T ; a = silu(g)*u ;
d = a @ Wd.T, with x:[2,2048,4096] f32, Wg/Wu:[14336,4096], Wd:[4096,14336].

Sharding: core c owns intermediate slice I_c = c*1792:(c+1)*1792. Each core
computes gT/uT/aT for its slice against all 4096 tokens, then a partial
dT[c] = WdT[I_c,:].T-contraction. Host sums the 8 partials (the tp_reduce)
and transposes back.

On-chip layout (everything transposed so contractions land on partitions):
  xT  [H=4096, T=4096] bf16            (rhs for gate/up)
  wg/wu [14, 128, 4096] bf16 pre-tiled (lhsT [k128, i128] stationary;
                                        wg[i, p, k*128+m] = Wg.T[k*128+p, i*128+m])
  wd  [32, 128, 1792] bf16 pre-tiled   (lhsT [i128, h128] stationary)
  out [H, T] f32 partial               (dT; host reduces + transposes)

The kernel is PE-bound at the bf16 roofline (~2.29ms of pure streaming), so
the schedule focuses on eliminating PE idle time:

  * Ramp: q0 is DMA-bound while its 8MB of xt tiles and 4MB of i0/i1
    weights stream in; measured, the sync HWDGE queue sustains ~2x the
    scalar queue's rate on this mix.  So sync carries the xt stream in
    k order (plus the wu chunks at just-in-time positions and i2's full
    tiles), scalar carries the wg chunks finest-first, and the PE ramp
    interleaves the i=0 and i=1 gate+up accumulations over the k loop
    (i1 lagged by 8 k-steps for weight-arrival slack) — 8 matmuls
    (~1.7us) of PE work per arriving xt tile, matching the two-queue
    delivery rate.  i=0 uses the pg/pu banks; i=1 borrows four
    single-bank tiles from the down-projection PSUM pool (idle during
    the gate phase).

  * Steady state: weights stream on scalar as full tiles (8KB lines,
    fast), xt and wd on sync, outputs on scalar.  The next q-block's
    first gate/up weights are hoisted ahead of the down loop so they
    never queue behind output traffic.

  * Down projection: n-outer/i-inner against single-bank PSUM tiles;
    each 512-column chunk is copied to SBUF and DMA'd on scalar while
    the next chunk accumulates.  The final h-tile's chunks drain on
    both HWDGE queues (sync is idle by then) in 256-column pieces so
    the end-of-kernel serial drain is minimal.
"""

import sys

if "/opt/trn_rl_repo" not in sys.path:
    sys.path.insert(0, "/opt/trn_rl_repo")

import numpy as np
import ml_dtypes

H = 4096          # hidden
I_FULL = 14336    # intermediate
T = 4096          # tokens (2*2048)
NCORES = 8
ISH = I_FULL // NCORES   # 1792 per-core intermediate slice
P = 128
QT = 1024         # tokens per outer block
NQ = T // QT      # 4
KT = H // P       # 32 contraction tiles for gate/up
IT = ISH // P     # 14 contraction tiles for down
HT = H // P       # 32 output-row tiles for down
NF = 512          # matmul moving free-dim (one PSUM bank of f32)

_BUILT = {}


def _build():
    if "nc" in _BUILT:
        return _BUILT["nc"]
    from concourse import bacc
    import concourse.mybir as mybir
    import concourse.tile as tile
    from contextlib import ExitStack

    bf = mybir.dt.bfloat16
    f32 = mybir.dt.float32
    nc = bacc.Bacc(
        "TRN2",
        target_bir_lowering=False,
        debug=False,
        enable_asserts=False,
        num_devices=NCORES,
    )

    xT = nc.dram_tensor("xT", [H, T], bf, kind="ExternalInput").ap()
    wg = nc.dram_tensor("wg", [IT, P, KT * P], bf, kind="ExternalInput").ap()
    wu = nc.dram_tensor("wu", [IT, P, KT * P], bf, kind="ExternalInput").ap()
    wd = nc.dram_tensor("wd", [HT, P, IT * P], bf, kind="ExternalInput").ap()
    out = nc.dram_tensor("out", [H, T], f32, kind="ExternalOutput").ap()

    # [p, k, t] view: per-partition rows stay contiguous in t
    x_r = xT.rearrange("(k p) t -> p k t", p=P)     # [128, 32, 4096]

    with tile.TileContext(nc) as tc, ExitStack() as ctx:
        xt_pool = ctx.enter_context(tc.tile_pool(name="xt", bufs=KT + 6))
        wg_pool = ctx.enter_context(tc.tile_pool(name="wg", bufs=3))
        wu_pool = ctx.enter_context(tc.tile_pool(name="wu", bufs=3))
        wd_pool = ctx.enter_context(tc.tile_pool(name="wd", bufs=6))
        at_pool = ctx.enter_context(tc.tile_pool(name="at", bufs=IT + 1))
        tmp_pool = ctx.enter_context(tc.tile_pool(name="tmp", bufs=2))
        dst_pool = ctx.enter_context(tc.tile_pool(name="dst", bufs=4))
        pg_pool = ctx.enter_context(tc.tile_pool(name="pg", bufs=1, space="PSUM"))
        pu_pool = ctx.enter_context(tc.tile_pool(name="pu", bufs=1, space="PSUM"))
        pd_pool = ctx.enter_context(tc.tile_pool(name="pd", bufs=4, space="PSUM"))

        def load_w(pool, src, i, tag):
            t = pool.tile([P, KT, P], bf, name=f"w_{tag}{i}", tag=tag)
            # src[i] is [128, 4096] contiguous per partition (8KB lines)
            nc.scalar.dma_start(
                out=t[:], in_=src[i].rearrange("p (k m) -> p k m", m=P)
            )
            return t

        wg_next = wu_next = None
        for q in range(NQ):
            t0 = q * QT

            ats = []
            i_start = 0
            if q == 0:
                # ---- kernel-start ramp (see module docstring) ----
                wg_t = wg_pool.tile([P, KT, P], bf, tag="wg")
                wu_t = wu_pool.tile([P, KT, P], bf, tag="wu")
                wg_t1 = wg_pool.tile([P, KT, P], bf, tag="wg")
                wu_t1 = wu_pool.tile([P, KT, P], bf, tag="wu")
                wv = {
                    0: (wg_t, wg[0].rearrange("p (k m) -> p k m", m=P)),
                    1: (wu_t, wu[0].rearrange("p (k m) -> p k m", m=P)),
                    2: (wg_t1, wg[1].rearrange("p (k m) -> p k m", m=P)),
                    3: (wu_t1, wu[1].rearrange("p (k m) -> p k m", m=P)),
                }
                xts = [
                    xt_pool.tile([P, QT], bf, name=f"xt{k}", tag="xt")
                    for k in range(KT)
                ]

                def wh(eng, wi, half):
                    t, v = wv[wi]
                    hs = slice(half * (KT // 2), (half + 1) * (KT // 2))
                    eng.dma_start(out=t[:, hs, :], in_=v[:, hs, :])

                def xl(eng, k):
                    eng.dma_start(out=xts[k][:], in_=x_r[:, k, t0 : t0 + QT])

                def wc(eng, wi, k0, k1):
                    t, v = wv[wi]
                    eng.dma_start(out=t[:, k0:k1, :], in_=v[:, k0:k1, :])

                # sync (~2x scalar queue rate, measured): all xt tiles in k
                # order, with the wu halves slotted just-in-time; scalar
                # gets the wg chunks (finest first, so the PE's k=0 matmuls
                # gate on ~128KB instead of 512KB)
                # xt0 in halves so the first matmul gates on 128KB
                nc.sync.dma_start(out=xts[0][:, 0:NF], in_=x_r[:, 0, t0 : t0 + NF])
                nc.sync.dma_start(
                    out=xts[0][:, NF:QT], in_=x_r[:, 0, t0 + NF : t0 + QT]
                )
                wc(nc.sync, 1, 0, 16)    # wu0 k0-15
                xl(nc.sync, 1)
                wc(nc.sync, 3, 0, 16)    # wu1 k0-15
                for k in range(2, 14):
                    xl(nc.sync, k)
                wc(nc.sync, 1, 16, KT)   # wu0 k16-31
                wc(nc.sync, 3, 16, KT)   # wu1 k16-31
                for k in range(14, KT):
                    xl(nc.sync, k)
                wc(nc.scalar, 0, 0, 4)   # wg0 k0-3
                wc(nc.scalar, 2, 0, 4)   # wg1 k0-3
                wc(nc.scalar, 0, 4, 16)  # wg0 k4-15
                wc(nc.scalar, 2, 4, 16)  # wg1 k4-15
                wc(nc.scalar, 0, 16, KT)  # wg0 k16-31
                wc(nc.scalar, 2, 16, KT)  # wg1 k16-31
                # i2's full weight tiles ride the fast sync queue behind the
                # xt stream so i2 never waits on the scalar backlog
                wg_t2 = wg_pool.tile([P, KT, P], bf, tag="wg")
                wu_t2 = wu_pool.tile([P, KT, P], bf, tag="wu")
                nc.sync.dma_start(
                    out=wg_t2[:], in_=wg[2].rearrange("p (k m) -> p k m", m=P)
                )
                nc.sync.dma_start(
                    out=wu_t2[:], in_=wu[2].rearrange("p (k m) -> p k m", m=P)
                )

                # ramp compute: i0 and i1 interleaved over k
                pg0 = pg_pool.tile([P, QT], f32, tag="pg")
                pu0 = pu_pool.tile([P, QT], f32, tag="pu")
                pg1n = [
                    pd_pool.tile([P, NF], f32, name=f"pg1n{n}", tag="pd")
                    for n in range(2)
                ]
                pu1n = [
                    pd_pool.tile([P, NF], f32, name=f"pu1n{n}", tag="pd")
                    for n in range(2)
                ]
                # i1 lags i0 by a few k-steps so its weight stream has
                # arrival slack at the very start of the kernel
                LAG = 8
                for kk in range(KT + LAG):
                    if kk < KT:
                        k = kk
                        st, sp = (k == 0), (k == KT - 1)
                        for n in range(QT // NF):
                            ns = slice(n * NF, (n + 1) * NF)
                            nc.tensor.matmul(
                                pg0[:, ns], wg_t[:, k, :], xts[k][:, ns],
                                start=st, stop=sp,
                            )
                        for n in range(QT // NF):
                            ns = slice(n * NF, (n + 1) * NF)
                            nc.tensor.matmul(
                                pu0[:, ns], wu_t[:, k, :], xts[k][:, ns],
                                start=st, stop=sp,
                            )
                    if kk >= LAG:
                        k = kk - LAG
                        st, sp = (k == 0), (k == KT - 1)
                        for n in range(QT // NF):
                            ns = slice(n * NF, (n + 1) * NF)
                            nc.tensor.matmul(
                                pg1n[n][:], wg_t1[:, k, :], xts[k][:, ns],
                                start=st, stop=sp,
                            )
                        for n in range(QT // NF):
                            ns = slice(n * NF, (n + 1) * NF)
                            nc.tensor.matmul(
                                pu1n[n][:], wu_t1[:, k, :], xts[k][:, ns],
                                start=st, stop=sp,
                            )
                tmp0 = tmp_pool.tile([P, QT], bf, tag="tmp")
                nc.scalar.activation(
                    tmp0[:], pg0[:], mybir.ActivationFunctionType.Silu
                )
                at0 = at_pool.tile([P, QT], bf, tag="at")
                nc.vector.tensor_tensor(
                    at0[:], tmp0[:], pu0[:], mybir.AluOpType.mult
                )
                tmp1 = tmp_pool.tile([P, QT], bf, tag="tmp")
                at1 = at_pool.tile([P, QT], bf, tag="at")
                for n in range(QT // NF):
                    ns = slice(n * NF, (n + 1) * NF)
                    nc.scalar.activation(
                        tmp1[:, ns], pg1n[n][:], mybir.ActivationFunctionType.Silu
                    )
                    nc.vector.tensor_tensor(
                        at1[:, ns], tmp1[:, ns], pu1n[n][:], mybir.AluOpType.mult
                    )
                ats += [at0, at1]
                i_start = 2
            else:
                # first weights were hoisted ahead of the previous down loop
                wg_t, wu_t = wg_next, wu_next
                xts = []
                for k in range(KT):
                    xt_t = xt_pool.tile([P, QT], bf, tag="xt")
                    nc.sync.dma_start(out=xt_t[:], in_=x_r[:, k, t0 : t0 + QT])
                    xts.append(xt_t)

            # ---- gate/up + silu*mul, producing aT[i] tiles ----
            for i in range(i_start, IT):
                if q == 0 and i == 2:
                    wg_t, wu_t = wg_t2, wu_t2
                elif i > 0:
                    wg_t = load_w(wg_pool, wg, i, "wg")
                    wu_t = load_w(wu_pool, wu, i, "wu")
                pg = pg_pool.tile([P, QT], f32, tag="pg")
                for k in range(KT):
                    for n in range(QT // NF):
                        nc.tensor.matmul(
                            pg[:, n * NF : (n + 1) * NF],
                            wg_t[:, k, :],
                            xts[k][:, n * NF : (n + 1) * NF],
                            start=(k == 0),
                            stop=(k == KT - 1),
                        )
                # silu(g) on ScalarE while the u matmuls run
                tmp = tmp_pool.tile([P, QT], bf, tag="tmp")
                nc.scalar.activation(
                    tmp[:], pg[:], mybir.ActivationFunctionType.Silu
                )
                pu = pu_pool.tile([P, QT], f32, tag="pu")
                for k in range(KT):
                    for n in range(QT // NF):
                        nc.tensor.matmul(
                            pu[:, n * NF : (n + 1) * NF],
                            wu_t[:, k, :],
                            xts[k][:, n * NF : (n + 1) * NF],
                            start=(k == 0),
                            stop=(k == KT - 1),
                        )
                at = at_pool.tile([P, QT], bf, tag="at")
                nc.vector.tensor_tensor(
                    at[:], tmp[:], pu[:], mybir.AluOpType.mult
                )
                ats.append(at)

            # hoist the next q-block's first gate/up weights ahead of the
            # down-phase output traffic on the scalar queue
            if q < NQ - 1:
                wg_next = load_w(wg_pool, wg, 0, "wg")
                wu_next = load_w(wu_pool, wu, 0, "wu")

            # ---- down projection: dT[h, t] partial ----
            # n-outer: each 512-col chunk accumulates into its own PSUM bank,
            # is copied to SBUF while the next chunk's matmuls run, and DMAs
            # out while later chunks compute
            for h in range(HT):
                h0 = h * P
                wd_t = wd_pool.tile([P, IT, P], bf, tag="wd")
                nc.sync.dma_start(
                    out=wd_t[:], in_=wd[h].rearrange("p (i m) -> p i m", m=P)
                )
                for n in range(QT // NF):
                    ns = slice(n * NF, (n + 1) * NF)
                    pd = pd_pool.tile([P, NF], f32, tag="pd")
                    for i in range(IT):
                        nc.tensor.matmul(
                            pd[:],
                            wd_t[:, i, :],
                            ats[i][:, ns],
                            start=(i == 0),
                            stop=(i == IT - 1),
                        )
                    last = q == NQ - 1 and h == HT - 1
                    if last and n == QT // NF - 1:
                        # final chunk: split the copy and DMA across both
                        # HWDGE queues so the end-of-kernel drain overlaps
                        hn = NF // 2
                        for c, ceng in ((0, nc.scalar), (1, nc.sync)):
                            dst = dst_pool.tile([P, hn], f32, tag="dstf")
                            nc.vector.tensor_copy(dst[:], pd[:, c * hn : (c + 1) * hn])
                            ceng.dma_start(
                                out=out[
                                    h0 : h0 + P,
                                    t0 + n * NF + c * hn : t0 + n * NF + (c + 1) * hn,
                                ],
                                in_=dst[:],
                            )
                    else:
                        dst = dst_pool.tile([P, NF], f32, tag="dst")
                        nc.vector.tensor_copy(dst[:], pd[:])
                        eng = nc.sync if last else nc.scalar
                        eng.dma_start(
                            out=out[h0 : h0 + P, t0 + n * NF : t0 + (n + 1) * NF],
                            in_=dst[:],
                        )

    nc.compile()
    _BUILT["nc"] = nc
    return nc


def _prep_inputs(x, Wg, Wu, Wd):
    bf = ml_dtypes.bfloat16
    xTn = x.reshape(T, H).T.astype(bf, order="C")        # [H, T]
    # single-pass cast + shard + pre-tile:
    #   wg[c][i, p, k*128+m] = Wg.T[k*128+p, c*1792 + i*128+m]
    wg_all = np.ascontiguousarray(
        Wg.reshape(NCORES, IT, P, KT, P).transpose(0, 1, 4, 3, 2), dtype=bf
    ).reshape(NCORES, IT, P, KT * P)
    wu_all = np.ascontiguousarray(
        Wu.reshape(NCORES, IT, P, KT, P).transpose(0, 1, 4, 3, 2), dtype=bf
    ).reshape(NCORES, IT, P, KT * P)
    #   wd[c][h, p, i*128+m] = Wd.T[c*1792 + i*128+p, h*128+m]
    wd_all = np.ascontiguousarray(
        Wd.reshape(HT, P, NCORES, IT, P).transpose(2, 0, 4, 3, 1), dtype=bf
    ).reshape(NCORES, HT, P, IT * P)
    return [
        {"xT": xTn, "wg": wg_all[c], "wu": wu_all[c], "wd": wd_all[c]}
        for c in range(NCORES)
    ]


def _run(in_maps, **kw):
    from concourse.bass_utils import run_bass_kernel_spmd

    nc = _build()
    return run_bass_kernel_spmd(nc, in_maps, core_ids=list(range(NCORES)), **kw)


def _gather(results, batch_shape):
    acc = results[0]["out"].astype(np.float32)
    for r in results[1:]:
        acc += r["out"]
    return np.ascontiguousarray(acc.T).reshape(batch_shape)


def kernel(x, Wg, Wu, Wd):
    x = np.asarray(x)
    in_maps = _prep_inputs(
        np.asarray(x, dtype=np.float32),
        np.asarray(Wg, dtype=np.float32),
        np.asarray(Wu, dtype=np.float32),
        np.asarray(Wd, dtype=np.float32),
    )
    res = _run(in_maps)
    return _gather(res.results, x.shape)



# revision 2
# speedup vs baseline: 1.0118x; 1.0118x over previous
"""Gated MLP (SwiGLU) on 8 TRN2 NeuronCores, tensor-parallel over the
intermediate dimension.

Math (per reference): g = x @ Wg.T ; u = x @ Wu.T ; a = silu(g)*u ;
d = a @ Wd.T, with x:[2,2048,4096] f32, Wg/Wu:[14336,4096], Wd:[4096,14336].

Sharding: core c owns intermediate slice I_c = c*1792:(c+1)*1792. Each core
computes gT/uT/aT for its slice against all 4096 tokens, then a partial
dT[c] = WdT[I_c,:].T-contraction. Host sums the 8 partials (the tp_reduce)
and transposes back.

On-chip layout (everything transposed so contractions land on partitions):
  xT  [H=4096, T=4096] bf16            (rhs for gate/up)
  wg/wu [14, 128, 4096] bf16 pre-tiled (lhsT [k128, i128] stationary;
                                        wg[i, p, k*128+m] = Wg.T[k*128+p, i*128+m])
  wd  [32, 128, 1792] bf16 pre-tiled   (lhsT [i128, h128] stationary)
  out [H, T] f32 partial               (dT; host reduces + transposes)

The kernel is PE-bound at the bf16 roofline: 10752 N=512 matmuls at the
warm issue gap of ~215.8ns = 2.32ms of pure streaming.  Trace analysis of
the previous schedule shows the steady state already runs gap-free at that
floor; the remaining overhead is the kernel-start ramp (~23us: DMA queue
startup, cold-HAM matmuls, weight/xt arrival stalls) and the end-of-kernel
drain.  This schedule attacks those:

  * PE pre-warm: the HAM clock gate holds the PE at 1.2GHz until it has
    seen ~3.4us of sustained matmul activity.  While the first DMAs are
    still in flight (payload cannot arrive before ~11us: ~9us of queue
    startup + transfer), we issue 10 dummy matmuls on a memset scratch
    tile into the first PSUM bank.  They warm the clock gate during
    otherwise-idle time so the real ramp starts at 2.4GHz.

  * LAG=0 ramp: q0 interleaves i=0 and i=1 gate+up per k step (8 matmuls
    per arriving 256KB xt tile + 128KB of weight chunks = 222GB/s demand,
    matching the ~250GB/s the two HWDGE queues deliver).  The previous
    schedule lagged i1 by 8 k-steps, which made the first 8 steps consume
    xt at 4 matmuls/tile = 370GB/s -> unavoidable stalls.  PSUM: i0 uses
    the pg/pu 2-bank tiles, i1 borrows the four single-bank down-phase
    tiles (idle during the gate phase).

  * JIT chunk schedule: weight chunks are split finest-first and placed
    on the two queues in need-order (sync ~1.7x the scalar rate, so sync
    carries xt + wu k0-15 + wu2, scalar carries wg + wu k16-31 + wg2 and
    the per-i steady-state loads).  The first chunks are 32KB so the
    first matmul gates on minimal data.

  * Steady state: weights stream on scalar as full tiles, xt on sync,
    outputs on scalar.  The next q-block's first gate/up weights are
    hoisted ahead of the down loop.  This part was measured at the issue
    floor and is unchanged.

  * Down projection: n-outer/i-inner against single-bank PSUM tiles;
    each 512-column chunk is copied to SBUF and DMA'd on scalar while
    the next chunk accumulates.  The final chunk drains as 4 128-column
    pieces, copies alternating between the vector and scalar engines and
    DMAs alternating between both HWDGE queues, so the end-of-kernel
    serial drain is minimal.
"""

import sys

if "/opt/trn_rl_repo" not in sys.path:
    sys.path.insert(0, "/opt/trn_rl_repo")

import numpy as np
import ml_dtypes

H = 4096          # hidden
I_FULL = 14336    # intermediate
T = 4096          # tokens (2*2048)
NCORES = 8
ISH = I_FULL // NCORES   # 1792 per-core intermediate slice
P = 128
QT = 1024         # tokens per outer block
NQ = T // QT      # 4
KT = H // P       # 32 contraction tiles for gate/up
IT = ISH // P     # 14 contraction tiles for down
HT = H // P       # 32 output-row tiles for down
NF = 512          # matmul moving free-dim (one PSUM bank of f32)
NWARM = 10        # PE pre-warm dummy matmuls (HAM clock-gate release)

_BUILT = {}


def _build():
    if "nc" in _BUILT:
        return _BUILT["nc"]
    from concourse import bacc
    import concourse.mybir as mybir
    import concourse.tile as tile
    from contextlib import ExitStack

    bf = mybir.dt.bfloat16
    f32 = mybir.dt.float32
    nc = bacc.Bacc(
        "TRN2",
        target_bir_lowering=False,
        debug=False,
        enable_asserts=False,
        num_devices=NCORES,
    )

    xT = nc.dram_tensor("xT", [H, T], bf, kind="ExternalInput").ap()
    wg = nc.dram_tensor("wg", [IT, P, KT * P], bf, kind="ExternalInput").ap()
    wu = nc.dram_tensor("wu", [IT, P, KT * P], bf, kind="ExternalInput").ap()
    wd = nc.dram_tensor("wd", [HT, P, IT * P], bf, kind="ExternalInput").ap()
    out = nc.dram_tensor("out", [H, T], f32, kind="ExternalOutput").ap()

    # [p, k, t] view: per-partition rows stay contiguous in t
    x_r = xT.rearrange("(k p) t -> p k t", p=P)     # [128, 32, 4096]

    with tile.TileContext(nc) as tc, ExitStack() as ctx:
        warm_pool = ctx.enter_context(tc.tile_pool(name="warm", bufs=1))
        xt_pool = ctx.enter_context(tc.tile_pool(name="xt", bufs=KT + 6))
        wg_pool = ctx.enter_context(tc.tile_pool(name="wg", bufs=3))
        wu_pool = ctx.enter_context(tc.tile_pool(name="wu", bufs=3))
        wd_pool = ctx.enter_context(tc.tile_pool(name="wd", bufs=6))
        at_pool = ctx.enter_context(tc.tile_pool(name="at", bufs=IT + 1))
        tmp_pool = ctx.enter_context(tc.tile_pool(name="tmp", bufs=2))
        dst_pool = ctx.enter_context(tc.tile_pool(name="dst", bufs=6))
        pg_pool = ctx.enter_context(tc.tile_pool(name="pg", bufs=1, space="PSUM"))
        pu_pool = ctx.enter_context(tc.tile_pool(name="pu", bufs=1, space="PSUM"))
        pd_pool = ctx.enter_context(tc.tile_pool(name="pd", bufs=4, space="PSUM"))

        def load_w(pool, src, i, tag, eng=None):
            t = pool.tile([P, KT, P], bf, name=f"w_{tag}{i}", tag=tag)
            # src[i] is [128, 4096] contiguous per partition (8KB lines)
            (eng or nc.scalar).dma_start(
                out=t[:], in_=src[i].rearrange("p (k m) -> p k m", m=P)
            )
            return t

        wg_next = wu_next = None
        for q in range(NQ):
            t0 = q * QT

            ats = []
            i_start = 0
            if q == 0:
                # ---- kernel-start ramp (see module docstring) ----
                # PE pre-warm: dummy matmuls on a memset scratch tile while
                # the first data DMAs are in flight.
                wt = warm_pool.tile([P, P + NF], bf, tag="warm")
                nc.gpsimd.memset(wt[:], 0)
                pg0 = pg_pool.tile([P, QT], f32, tag="pg")
                for _ in range(NWARM):
                    nc.tensor.matmul(
                        pg0[:, 0:NF], wt[:, 0:P], wt[:, P : P + NF],
                        start=True, stop=True,
                    )

                wg_t = wg_pool.tile([P, KT, P], bf, tag="wg")
                wu_t = wu_pool.tile([P, KT, P], bf, tag="wu")
                wg_t1 = wg_pool.tile([P, KT, P], bf, tag="wg")
                wu_t1 = wu_pool.tile([P, KT, P], bf, tag="wu")
                wv = {
                    0: (wg_t, wg[0].rearrange("p (k m) -> p k m", m=P)),
                    1: (wu_t, wu[0].rearrange("p (k m) -> p k m", m=P)),
                    2: (wg_t1, wg[1].rearrange("p (k m) -> p k m", m=P)),
                    3: (wu_t1, wu[1].rearrange("p (k m) -> p k m", m=P)),
                }
                xts = [
                    xt_pool.tile([P, QT], bf, name=f"xt{k}", tag="xt")
                    for k in range(KT)
                ]

                def xl(eng, k):
                    eng.dma_start(out=xts[k][:], in_=x_r[:, k, t0 : t0 + QT])

                def wc(eng, wi, k0, k1):
                    t, v = wv[wi]
                    eng.dma_start(out=t[:, k0:k1, :], in_=v[:, k0:k1, :])

                # sync queue (~1.7x the scalar rate on this mix): the xt
                # stream in k order with the wu0/wu1 k0-15 chunks slotted
                # just-in-time; wu2's full tile trails the stream.
                nc.sync.dma_start(out=xts[0][:, 0:NF], in_=x_r[:, 0, t0 : t0 + NF])
                nc.sync.dma_start(
                    out=xts[0][:, NF:QT], in_=x_r[:, 0, t0 + NF : t0 + QT]
                )
                wc(nc.sync, 1, 0, 1)      # wu0 k0
                wc(nc.sync, 3, 0, 1)      # wu1 k0
                xl(nc.sync, 1)
                wc(nc.sync, 1, 1, 3)      # wu0 k1-2
                wc(nc.sync, 3, 1, 3)      # wu1 k1-2
                xl(nc.sync, 2)
                wc(nc.sync, 1, 3, 6)      # wu0 k3-5
                wc(nc.sync, 3, 3, 6)      # wu1 k3-5
                xl(nc.sync, 3)
                wc(nc.sync, 1, 6, 10)     # wu0 k6-9
                wc(nc.sync, 3, 6, 10)     # wu1 k6-9
                xl(nc.sync, 4)
                xl(nc.sync, 5)
                wc(nc.sync, 1, 10, 16)    # wu0 k10-15
                wc(nc.sync, 3, 10, 16)    # wu1 k10-15
                for k in range(6, KT):
                    xl(nc.sync, k)
                # scalar queue: wg0/wg1 chunks finest-first in need order,
                # then the k16-31 wu chunks (needed a full k-pass later).
                wc(nc.scalar, 0, 0, 1)    # wg0 k0
                wc(nc.scalar, 2, 0, 1)    # wg1 k0
                wc(nc.scalar, 0, 1, 3)    # wg0 k1-2
                wc(nc.scalar, 2, 1, 3)    # wg1 k1-2
                wc(nc.scalar, 0, 3, 6)    # wg0 k3-5
                wc(nc.scalar, 2, 3, 6)    # wg1 k3-5
                wc(nc.scalar, 0, 6, 10)   # wg0 k6-9
                wc(nc.scalar, 2, 6, 10)   # wg1 k6-9
                wc(nc.scalar, 0, 10, 16)  # wg0 k10-15
                wc(nc.scalar, 2, 10, 16)  # wg1 k10-15
                wc(nc.scalar, 0, 16, 24)  # wg0 k16-23
                wc(nc.scalar, 2, 16, 24)  # wg1 k16-23
                wc(nc.scalar, 1, 16, 24)  # wu0 k16-23
                wc(nc.scalar, 3, 16, 24)  # wu1 k16-23
                wc(nc.scalar, 0, 24, KT)  # wg0 k24-31
                wc(nc.scalar, 2, 24, KT)  # wg1 k24-31
                wc(nc.scalar, 1, 24, KT)  # wu0 k24-31
                wc(nc.scalar, 3, 24, KT)  # wu1 k24-31
                # i2's full weight tiles: wu2 rides sync behind the xt
                # stream, wg2 rides scalar behind the ramp chunks.
                wg_t2 = wg_pool.tile([P, KT, P], bf, tag="wg")
                wu_t2 = wu_pool.tile([P, KT, P], bf, tag="wu")
                nc.scalar.dma_start(
                    out=wg_t2[:], in_=wg[2].rearrange("p (k m) -> p k m", m=P)
                )
                nc.sync.dma_start(
                    out=wu_t2[:], in_=wu[2].rearrange("p (k m) -> p k m", m=P)
                )

                # ramp compute: i0 and i1 in lockstep over k (8 matmuls per
                # xt tile keeps the DMA demand at ~222GB/s from step 0)
                pu0 = pu_pool.tile([P, QT], f32, tag="pu")
                pg1n = [
                    pd_pool.tile([P, NF], f32, name=f"pg1n{n}", tag="pd")
                    for n in range(2)
                ]
                pu1n = [
                    pd_pool.tile([P, NF], f32, name=f"pu1n{n}", tag="pd")
                    for n in range(2)
                ]
                for k in range(KT):
                    st, sp = (k == 0), (k == KT - 1)
                    for n in range(QT // NF):
                        ns = slice(n * NF, (n + 1) * NF)
                        nc.tensor.matmul(
                            pg0[:, ns], wg_t[:, k, :], xts[k][:, ns],
                            start=st, stop=sp,
                        )
                    for n in range(QT // NF):
                        ns = slice(n * NF, (n + 1) * NF)
                        nc.tensor.matmul(
                            pu0[:, ns], wu_t[:, k, :], xts[k][:, ns],
                            start=st, stop=sp,
                        )
                    for n in range(QT // NF):
                        ns = slice(n * NF, (n + 1) * NF)
                        nc.tensor.matmul(
                            pg1n[n][:], wg_t1[:, k, :], xts[k][:, ns],
                            start=st, stop=sp,
                        )
                    for n in range(QT // NF):
                        ns = slice(n * NF, (n + 1) * NF)
                        nc.tensor.matmul(
                            pu1n[n][:], wu_t1[:, k, :], xts[k][:, ns],
                            start=st, stop=sp,
                        )
                tmp0 = tmp_pool.tile([P, QT], bf, tag="tmp")
                nc.scalar.activation(
                    tmp0[:], pg0[:], mybir.ActivationFunctionType.Silu
                )
                at0 = at_pool.tile([P, QT], bf, tag="at")
                nc.vector.tensor_tensor(
                    at0[:], tmp0[:], pu0[:], mybir.AluOpType.mult
                )
                tmp1 = tmp_pool.tile([P, QT], bf, tag="tmp")
                at1 = at_pool.tile([P, QT], bf, tag="at")
                for n in range(QT // NF):
                    ns = slice(n * NF, (n + 1) * NF)
                    nc.scalar.activation(
                        tmp1[:, ns], pg1n[n][:], mybir.ActivationFunctionType.Silu
                    )
                    nc.vector.tensor_tensor(
                        at1[:, ns], tmp1[:, ns], pu1n[n][:], mybir.AluOpType.mult
                    )
                ats += [at0, at1]
                i_start = 2
            else:
                # first weights were hoisted ahead of the previous down loop
                wg_t, wu_t = wg_next, wu_next
                xts = []
                for k in range(KT):
                    xt_t = xt_pool.tile([P, QT], bf, tag="xt")
                    nc.sync.dma_start(out=xt_t[:], in_=x_r[:, k, t0 : t0 + QT])
                    xts.append(xt_t)

            # ---- gate/up + silu*mul, producing aT[i] tiles ----
            for i in range(i_start, IT):
                if q == 0 and i == 2:
                    wg_t, wu_t = wg_t2, wu_t2
                elif i > 0:
                    wg_t = load_w(wg_pool, wg, i, "wg")
                    wu_t = load_w(wu_pool, wu, i, "wu")
                pg = pg_pool.tile([P, QT], f32, tag="pg")
                for k in range(KT):
                    for n in range(QT // NF):
                        nc.tensor.matmul(
                            pg[:, n * NF : (n + 1) * NF],
                            wg_t[:, k, :],
                            xts[k][:, n * NF : (n + 1) * NF],
                            start=(k == 0),
                            stop=(k == KT - 1),
                        )
                # silu(g) on ScalarE while the u matmuls run
                tmp = tmp_pool.tile([P, QT], bf, tag="tmp")
                nc.scalar.activation(
                    tmp[:], pg[:], mybir.ActivationFunctionType.Silu
                )
                pu = pu_pool.tile([P, QT], f32, tag="pu")
                for k in range(KT):
                    for n in range(QT // NF):
                        nc.tensor.matmul(
                            pu[:, n * NF : (n + 1) * NF],
                            wu_t[:, k, :],
                            xts[k][:, n * NF : (n + 1) * NF],
                            start=(k == 0),
                            stop=(k == KT - 1),
                        )
                at = at_pool.tile([P, QT], bf, tag="at")
                nc.vector.tensor_tensor(
                    at[:], tmp[:], pu[:], mybir.AluOpType.mult
                )
                ats.append(at)

            # hoist the next q-block's first gate/up weights ahead of the
            # down-phase output traffic on the scalar queue
            if q < NQ - 1:
                wg_next = load_w(wg_pool, wg, 0, "wg")
                wu_next = load_w(wu_pool, wu, 0, "wu")

            # ---- down projection: dT[h, t] partial ----
            # n-outer: each 512-col chunk accumulates into its own PSUM bank,
            # is copied to SBUF while the next chunk's matmuls run, and DMAs
            # out while later chunks compute
            for h in range(HT):
                h0 = h * P
                wd_t = wd_pool.tile([P, IT, P], bf, tag="wd")
                nc.sync.dma_start(
                    out=wd_t[:], in_=wd[h].rearrange("p (i m) -> p i m", m=P)
                )
                for n in range(QT // NF):
                    ns = slice(n * NF, (n + 1) * NF)
                    pd = pd_pool.tile([P, NF], f32, tag="pd")
                    for i in range(IT):
                        nc.tensor.matmul(
                            pd[:],
                            wd_t[:, i, :],
                            ats[i][:, ns],
                            start=(i == 0),
                            stop=(i == IT - 1),
                        )
                    last = q == NQ - 1 and h == HT - 1
                    if last and n == QT // NF - 1:
                        # final chunk: drain as 4 128-col pieces, copies
                        # alternating vector/scalar engines, DMAs alternating
                        # both HWDGE queues, so the tail is minimal
                        hn = NF // 4
                        for c in range(4):
                            dst = dst_pool.tile([P, hn], f32, tag="dstf")
                            cs = slice(c * hn, (c + 1) * hn)
                            if c % 2 == 0:
                                nc.vector.tensor_copy(dst[:], pd[:, cs])
                            else:
                                nc.scalar.activation(
                                    dst[:], pd[:, cs],
                                    mybir.ActivationFunctionType.Copy,
                                )
                            ceng = nc.scalar if c % 2 == 0 else nc.sync
                            ceng.dma_start(
                                out=out[
                                    h0 : h0 + P,
                                    t0 + n * NF + c * hn : t0 + n * NF + (c + 1) * hn,
                                ],
                                in_=dst[:],
                            )
                    else:
                        dst = dst_pool.tile([P, NF], f32, tag="dst")
                        nc.vector.tensor_copy(dst[:], pd[:])
                        eng = nc.sync if last else nc.scalar
                        eng.dma_start(
                            out=out[h0 : h0 + P, t0 + n * NF : t0 + (n + 1) * NF],
                            in_=dst[:],
                        )

    nc.compile()
    _BUILT["nc"] = nc
    return nc


def _prep_inputs(x, Wg, Wu, Wd):
    bf = ml_dtypes.bfloat16
    xTn = x.reshape(T, H).T.astype(bf, order="C")        # [H, T]
    # single-pass cast + shard + pre-tile:
    #   wg[c][i, p, k*128+m] = Wg.T[k*128+p, c*1792 + i*128+m]
    wg_all = np.ascontiguousarray(
        Wg.reshape(NCORES, IT, P, KT, P).transpose(0, 1, 4, 3, 2), dtype=bf
    ).reshape(NCORES, IT, P, KT * P)
    wu_all = np.ascontiguousarray(
        Wu.reshape(NCORES, IT, P, KT, P).transpose(0, 1, 4, 3, 2), dtype=bf
    ).reshape(NCORES, IT, P, KT * P)
    #   wd[c][h, p, i*128+m] = Wd.T[c*1792 + i*128+p, h*128+m]
    wd_all = np.ascontiguousarray(
        Wd.reshape(HT, P, NCORES, IT, P).transpose(2, 0, 4, 3, 1), dtype=bf
    ).reshape(NCORES, HT, P, IT * P)
    return [
        {"xT": xTn, "wg": wg_all[c], "wu": wu_all[c], "wd": wd_all[c]}
        for c in range(NCORES)
    ]


def _run(in_maps, **kw):
    from concourse.bass_utils import run_bass_kernel_spmd

    nc = _build()
    return run_bass_kernel_spmd(nc, in_maps, core_ids=list(range(NCORES)), **kw)


def _gather(results, batch_shape):
    acc = results[0]["out"].astype(np.float32)
    for r in results[1:]:
        acc += r["out"]
    return np.ascontiguousarray(acc.T).reshape(batch_shape)


def kernel(x, Wg, Wu, Wd):
    x = np.asarray(x)
    in_maps = _prep_inputs(
        np.asarray(x, dtype=np.float32),
        np.asarray(Wg, dtype=np.float32),
        np.asarray(Wu, dtype=np.float32),
        np.asarray(Wd, dtype=np.float32),
    )
    res = _run(in_maps)
    return _gather(res.results, x.shape)


# revision 5
# speedup vs baseline: 1.0149x; 1.0032x over previous
"""Gated MLP (SwiGLU) on 8 TRN2 NeuronCores, tensor-parallel over the
intermediate dimension.

Math (per reference): g = x @ Wg.T ; u = x @ Wu.T ; a = silu(g)*u ;
d = a @ Wd.T, with x:[2,2048,4096] f32, Wg/Wu:[14336,4096], Wd:[4096,14336].

Sharding: core c owns intermediate slice I_c = c*1792:(c+1)*1792. Each core
computes gT/uT/aT for its slice against all 4096 tokens, then a partial
dT[c] = WdT[I_c,:].T-contraction. Host sums the 8 partials (the tp_reduce)
and transposes back.

On-chip layout (everything transposed so contractions land on partitions):
  xT  [H=4096, T=4096] bf16            (rhs for gate/up)
  wg/wu [14, 128, 4096] bf16 pre-tiled (lhsT [k128, i128] stationary;
                                        wg[i, p, k*128+m] = Wg.T[k*128+p, i*128+m])
  wd  [32, 128, 1792] bf16 pre-tiled   (lhsT [i128, h128] stationary)
  out [H, T] f32 partial               (dT; host reduces + transposes)

The kernel is PE-bound at the bf16 roofline: 10752 N=512 matmuls at the
warm issue gap of ~215.8ns = 2.32ms of pure streaming.  Trace analysis of
the previous schedule shows the steady state already runs gap-free at that
floor; the remaining overhead is the kernel-start ramp (~23us: DMA queue
startup, cold-HAM matmuls, weight/xt arrival stalls) and the end-of-kernel
drain.  This schedule attacks those:

  * PE pre-warm: the HAM clock gate holds the PE at 1.2GHz until it has
    seen ~3.4us of sustained matmul activity.  While the first DMAs are
    still in flight (payload cannot arrive before ~11us: ~9us of queue
    startup + transfer), we issue 10 dummy matmuls on a memset scratch
    tile into the first PSUM bank.  They warm the clock gate during
    otherwise-idle time so the real ramp starts at 2.4GHz.

  * LAG=0 ramp: q0 interleaves i=0 and i=1 gate+up per k step (8 matmuls
    per arriving 256KB xt tile + 128KB of weight chunks = 222GB/s demand,
    matching the ~250GB/s the two HWDGE queues deliver).  The previous
    schedule lagged i1 by 8 k-steps, which made the first 8 steps consume
    xt at 4 matmuls/tile = 370GB/s -> unavoidable stalls.  PSUM: i0 uses
    the pg/pu 2-bank tiles, i1 borrows the four single-bank down-phase
    tiles (idle during the gate phase).

  * JIT chunk schedule: weight chunks are split finest-first and placed
    on the two queues in need-order (sync ~1.7x the scalar rate, so sync
    carries xt + wu k0-15 + wu2, scalar carries wg + wu k16-31 + wg2 and
    the per-i steady-state loads).  The first chunks are 32KB so the
    first matmul gates on minimal data.

  * Steady state: weights stream on scalar as full tiles, xt on sync,
    outputs on scalar.  The next q-block's first gate/up weights are
    hoisted ahead of the down loop.  This part was measured at the issue
    floor and is unchanged.

  * Down projection: n-outer/i-inner against single-bank PSUM tiles;
    each 512-column chunk is copied to SBUF and DMA'd on scalar while
    the next chunk accumulates.  The final chunk drains as 4 128-column
    pieces, copies alternating between the vector and scalar engines and
    DMAs alternating between both HWDGE queues, so the end-of-kernel
    serial drain is minimal.
"""

import sys

if "/opt/trn_rl_repo" not in sys.path:
    sys.path.insert(0, "/opt/trn_rl_repo")

import numpy as np
import ml_dtypes

H = 4096          # hidden
I_FULL = 14336    # intermediate
T = 4096          # tokens (2*2048)
NCORES = 8
ISH = I_FULL // NCORES   # 1792 per-core intermediate slice
P = 128
QT = 1024         # tokens per outer block
NQ = T // QT      # 4
KT = H // P       # 32 contraction tiles for gate/up
IT = ISH // P     # 14 contraction tiles for down
HT = H // P       # 32 output-row tiles for down
NF = 512          # matmul moving free-dim (one PSUM bank of f32)
NWARM = 12        # PE pre-warm dummy matmuls (HAM clock-gate release)

_BUILT = {}


def _build():
    if "nc" in _BUILT:
        return _BUILT["nc"]
    from concourse import bacc
    import concourse.mybir as mybir
    import concourse.tile as tile
    from contextlib import ExitStack

    bf = mybir.dt.bfloat16
    f32 = mybir.dt.float32
    nc = bacc.Bacc(
        "TRN2",
        target_bir_lowering=False,
        debug=False,
        enable_asserts=False,
        num_devices=NCORES,
    )

    xT = nc.dram_tensor("xT", [H, T], bf, kind="ExternalInput").ap()
    wg = nc.dram_tensor("wg", [IT, P, KT * P], bf, kind="ExternalInput").ap()
    wu = nc.dram_tensor("wu", [IT, P, KT * P], bf, kind="ExternalInput").ap()
    wd = nc.dram_tensor("wd", [HT, P, IT * P], bf, kind="ExternalInput").ap()
    out = nc.dram_tensor("out", [H, T], f32, kind="ExternalOutput").ap()

    # [p, k, t] view: per-partition rows stay contiguous in t
    x_r = xT.rearrange("(k p) t -> p k t", p=P)     # [128, 32, 4096]

    with tile.TileContext(nc) as tc, ExitStack() as ctx:
        warm_pool = ctx.enter_context(tc.tile_pool(name="warm", bufs=1))
        xt_pool = ctx.enter_context(tc.tile_pool(name="xt", bufs=KT + 6))
        wg_pool = ctx.enter_context(tc.tile_pool(name="wg", bufs=3))
        wu_pool = ctx.enter_context(tc.tile_pool(name="wu", bufs=3))
        wd_pool = ctx.enter_context(tc.tile_pool(name="wd", bufs=6))
        at_pool = ctx.enter_context(tc.tile_pool(name="at", bufs=IT + 1))
        tmp_pool = ctx.enter_context(tc.tile_pool(name="tmp", bufs=2))
        dst_pool = ctx.enter_context(tc.tile_pool(name="dst", bufs=6))
        pg_pool = ctx.enter_context(tc.tile_pool(name="pg", bufs=1, space="PSUM"))
        pu_pool = ctx.enter_context(tc.tile_pool(name="pu", bufs=1, space="PSUM"))
        pd_pool = ctx.enter_context(tc.tile_pool(name="pd", bufs=4, space="PSUM"))

        def load_w(pool, src, i, tag, eng=None):
            t = pool.tile([P, KT, P], bf, name=f"w_{tag}{i}", tag=tag)
            # src[i] is [128, 4096] contiguous per partition (8KB lines)
            (eng or nc.scalar).dma_start(
                out=t[:], in_=src[i].rearrange("p (k m) -> p k m", m=P)
            )
            return t

        wg_next = wu_next = None
        for q in range(NQ):
            t0 = q * QT

            ats = []
            i_start = 0
            if q == 0:
                # ---- kernel-start ramp (see module docstring) ----
                # PE pre-warm: dummy matmuls on a memset scratch tile while
                # the first data DMAs are in flight.
                wt = warm_pool.tile([P, P + NF], bf, tag="warm")
                nc.gpsimd.memset(wt[:], 0)
                pg0 = pg_pool.tile([P, QT], f32, tag="pg")
                for _ in range(NWARM):
                    nc.tensor.matmul(
                        pg0[:, 0:NF], wt[:, 0:P], wt[:, P : P + NF],
                        start=True, stop=True,
                    )

                wg_t = wg_pool.tile([P, KT, P], bf, tag="wg")
                wu_t = wu_pool.tile([P, KT, P], bf, tag="wu")
                wg_t1 = wg_pool.tile([P, KT, P], bf, tag="wg")
                wu_t1 = wu_pool.tile([P, KT, P], bf, tag="wu")
                wv = {
                    0: (wg_t, wg[0].rearrange("p (k m) -> p k m", m=P)),
                    1: (wu_t, wu[0].rearrange("p (k m) -> p k m", m=P)),
                    2: (wg_t1, wg[1].rearrange("p (k m) -> p k m", m=P)),
                    3: (wu_t1, wu[1].rearrange("p (k m) -> p k m", m=P)),
                }
                xts = [
                    xt_pool.tile([P, QT], bf, name=f"xt{k}", tag="xt")
                    for k in range(KT)
                ]

                def xl(eng, k):
                    eng.dma_start(out=xts[k][:], in_=x_r[:, k, t0 : t0 + QT])

                def wc(eng, wi, k0, k1):
                    t, v = wv[wi]
                    eng.dma_start(out=t[:, k0:k1, :], in_=v[:, k0:k1, :])

                # sync queue (~1.7x the scalar rate on this mix): the xt
                # stream alone, in strict k order, so xt[k] is never behind
                # the per-k consumption pace; wu2's full tile trails it.
                nc.sync.dma_start(out=xts[0][:, 0:NF], in_=x_r[:, 0, t0 : t0 + NF])
                nc.sync.dma_start(
                    out=xts[0][:, NF:QT], in_=x_r[:, 0, t0 + NF : t0 + QT]
                )
                for k in range(1, KT):
                    xl(nc.sync, k)
                # scalar queue: all four weight streams, chunks per-k in
                # need order (wg before wu before the i1 pair), finest-first
                # so the early k steps gate on minimal data.
                for k0, k1 in ((0, 1), (1, 2), (2, 4), (4, 7), (7, 11),
                               (11, 16), (16, 24), (24, KT)):
                    wc(nc.scalar, 0, k0, k1)   # wg0
                    wc(nc.scalar, 1, k0, k1)   # wu0
                    wc(nc.scalar, 2, k0, k1)   # wg1
                    wc(nc.scalar, 3, k0, k1)   # wu1
                # i2's full weight tiles: wu2 rides sync behind the xt
                # stream, wg2 rides scalar behind the ramp chunks.
                wg_t2 = wg_pool.tile([P, KT, P], bf, tag="wg")
                wu_t2 = wu_pool.tile([P, KT, P], bf, tag="wu")
                nc.scalar.dma_start(
                    out=wg_t2[:], in_=wg[2].rearrange("p (k m) -> p k m", m=P)
                )
                nc.sync.dma_start(
                    out=wu_t2[:], in_=wu[2].rearrange("p (k m) -> p k m", m=P)
                )

                # ramp compute: i0 and i1 in lockstep over k (8 matmuls per
                # xt tile keeps the DMA demand at ~222GB/s from step 0)
                pu0 = pu_pool.tile([P, QT], f32, tag="pu")
                pg1n = [
                    pd_pool.tile([P, NF], f32, name=f"pg1n{n}", tag="pd")
                    for n in range(2)
                ]
                pu1n = [
                    pd_pool.tile([P, NF], f32, name=f"pu1n{n}", tag="pd")
                    for n in range(2)
                ]
                for k in range(KT):
                    st, sp = (k == 0), (k == KT - 1)
                    for n in range(QT // NF):
                        ns = slice(n * NF, (n + 1) * NF)
                        nc.tensor.matmul(
                            pg0[:, ns], wg_t[:, k, :], xts[k][:, ns],
                            start=st, stop=sp,
                        )
                    for n in range(QT // NF):
                        ns = slice(n * NF, (n + 1) * NF)
                        nc.tensor.matmul(
                            pu0[:, ns], wu_t[:, k, :], xts[k][:, ns],
                            start=st, stop=sp,
                        )
                    for n in range(QT // NF):
                        ns = slice(n * NF, (n + 1) * NF)
                        nc.tensor.matmul(
                            pg1n[n][:], wg_t1[:, k, :], xts[k][:, ns],
                            start=st, stop=sp,
                        )
                    for n in range(QT // NF):
                        ns = slice(n * NF, (n + 1) * NF)
                        nc.tensor.matmul(
                            pu1n[n][:], wu_t1[:, k, :], xts[k][:, ns],
                            start=st, stop=sp,
                        )
                tmp0 = tmp_pool.tile([P, QT], bf, tag="tmp")
                nc.scalar.activation(
                    tmp0[:], pg0[:], mybir.ActivationFunctionType.Silu
                )
                at0 = at_pool.tile([P, QT], bf, tag="at")
                nc.vector.tensor_tensor(
                    at0[:], tmp0[:], pu0[:], mybir.AluOpType.mult
                )
                tmp1 = tmp_pool.tile([P, QT], bf, tag="tmp")
                at1 = at_pool.tile([P, QT], bf, tag="at")
                for n in range(QT // NF):
                    ns = slice(n * NF, (n + 1) * NF)
                    nc.scalar.activation(
                        tmp1[:, ns], pg1n[n][:], mybir.ActivationFunctionType.Silu
                    )
                    nc.vector.tensor_tensor(
                        at1[:, ns], tmp1[:, ns], pu1n[n][:], mybir.AluOpType.mult
                    )
                ats += [at0, at1]
                i_start = 2
            else:
                # first weights were hoisted ahead of the previous down loop
                wg_t, wu_t = wg_next, wu_next
                xts = []
                for k in range(KT):
                    xt_t = xt_pool.tile([P, QT], bf, tag="xt")
                    nc.sync.dma_start(out=xt_t[:], in_=x_r[:, k, t0 : t0 + QT])
                    xts.append(xt_t)

            # ---- gate/up + silu*mul, producing aT[i] tiles ----
            for i in range(i_start, IT):
                if q == 0 and i == 2:
                    wg_t, wu_t = wg_t2, wu_t2
                elif i > 0:
                    wg_t = load_w(wg_pool, wg, i, "wg")
                    wu_t = load_w(wu_pool, wu, i, "wu")
                pg = pg_pool.tile([P, QT], f32, tag="pg")
                for k in range(KT):
                    for n in range(QT // NF):
                        nc.tensor.matmul(
                            pg[:, n * NF : (n + 1) * NF],
                            wg_t[:, k, :],
                            xts[k][:, n * NF : (n + 1) * NF],
                            start=(k == 0),
                            stop=(k == KT - 1),
                        )
                # silu(g) on ScalarE while the u matmuls run
                tmp = tmp_pool.tile([P, QT], bf, tag="tmp")
                nc.scalar.activation(
                    tmp[:], pg[:], mybir.ActivationFunctionType.Silu
                )
                pu = pu_pool.tile([P, QT], f32, tag="pu")
                for k in range(KT):
                    for n in range(QT // NF):
                        nc.tensor.matmul(
                            pu[:, n * NF : (n + 1) * NF],
                            wu_t[:, k, :],
                            xts[k][:, n * NF : (n + 1) * NF],
                            start=(k == 0),
                            stop=(k == KT - 1),
                        )
                at = at_pool.tile([P, QT], bf, tag="at")
                nc.vector.tensor_tensor(
                    at[:], tmp[:], pu[:], mybir.AluOpType.mult
                )
                ats.append(at)

            # hoist the next q-block's first gate/up weights ahead of the
            # down-phase output traffic on the scalar queue
            if q < NQ - 1:
                wg_next = load_w(wg_pool, wg, 0, "wg")
                wu_next = load_w(wu_pool, wu, 0, "wu")

            # ---- down projection: dT[h, t] partial ----
            # n-outer: each 512-col chunk accumulates into its own PSUM bank,
            # is copied to SBUF while the next chunk's matmuls run, and DMAs
            # out while later chunks compute
            for h in range(HT):
                h0 = h * P
                wd_t = wd_pool.tile([P, IT, P], bf, tag="wd")
                nc.sync.dma_start(
                    out=wd_t[:], in_=wd[h].rearrange("p (i m) -> p i m", m=P)
                )
                for n in range(QT // NF):
                    ns = slice(n * NF, (n + 1) * NF)
                    pd = pd_pool.tile([P, NF], f32, tag="pd")
                    for i in range(IT):
                        nc.tensor.matmul(
                            pd[:],
                            wd_t[:, i, :],
                            ats[i][:, ns],
                            start=(i == 0),
                            stop=(i == IT - 1),
                        )
                    last = q == NQ - 1 and h == HT - 1
                    if last and n == QT // NF - 1:
                        # final chunk: drain as 2 256-col pieces with the
                        # copies on the vector AND scalar engines in parallel
                        # and the DMAs on both HWDGE queues, so the
                        # end-of-kernel serial drain is minimal
                        hn = NF // 2
                        for c, ceng in ((0, nc.scalar), (1, nc.sync)):
                            dst = dst_pool.tile([P, hn], f32, tag="dstf")
                            cs = slice(c * hn, (c + 1) * hn)
                            if c == 0:
                                nc.vector.tensor_copy(dst[:], pd[:, cs])
                            else:
                                nc.scalar.activation(
                                    dst[:], pd[:, cs],
                                    mybir.ActivationFunctionType.Copy,
                                )
                            ceng.dma_start(
                                out=out[
                                    h0 : h0 + P,
                                    t0 + n * NF + c * hn : t0 + n * NF + (c + 1) * hn,
                                ],
                                in_=dst[:],
                            )
                    else:
                        dst = dst_pool.tile([P, NF], f32, tag="dst")
                        nc.vector.tensor_copy(dst[:], pd[:])
                        eng = nc.sync if last else nc.scalar
                        eng.dma_start(
                            out=out[h0 : h0 + P, t0 + n * NF : t0 + (n + 1) * NF],
                            in_=dst[:],
                        )

    nc.compile()
    _BUILT["nc"] = nc
    return nc


def _prep_inputs(x, Wg, Wu, Wd):
    bf = ml_dtypes.bfloat16
    xTn = x.reshape(T, H).T.astype(bf, order="C")        # [H, T]
    # single-pass cast + shard + pre-tile:
    #   wg[c][i, p, k*128+m] = Wg.T[k*128+p, c*1792 + i*128+m]
    wg_all = np.ascontiguousarray(
        Wg.reshape(NCORES, IT, P, KT, P).transpose(0, 1, 4, 3, 2), dtype=bf
    ).reshape(NCORES, IT, P, KT * P)
    wu_all = np.ascontiguousarray(
        Wu.reshape(NCORES, IT, P, KT, P).transpose(0, 1, 4, 3, 2), dtype=bf
    ).reshape(NCORES, IT, P, KT * P)
    #   wd[c][h, p, i*128+m] = Wd.T[c*1792 + i*128+p, h*128+m]
    wd_all = np.ascontiguousarray(
        Wd.reshape(HT, P, NCORES, IT, P).transpose(2, 0, 4, 3, 1), dtype=bf
    ).reshape(NCORES, HT, P, IT * P)
    return [
        {"xT": xTn, "wg": wg_all[c], "wu": wu_all[c], "wd": wd_all[c]}
        for c in range(NCORES)
    ]


def _run(in_maps, **kw):
    from concourse.bass_utils import run_bass_kernel_spmd

    nc = _build()
    return run_bass_kernel_spmd(nc, in_maps, core_ids=list(range(NCORES)), **kw)


def _gather(results, batch_shape):
    acc = results[0]["out"].astype(np.float32)
    for r in results[1:]:
        acc += r["out"]
    return np.ascontiguousarray(acc.T).reshape(batch_shape)


def kernel(x, Wg, Wu, Wd):
    x = np.asarray(x)
    in_maps = _prep_inputs(
        np.asarray(x, dtype=np.float32),
        np.asarray(Wg, dtype=np.float32),
        np.asarray(Wu, dtype=np.float32),
        np.asarray(Wd, dtype=np.float32),
    )
    res = _run(in_maps)
    return _gather(res.results, x.shape)
